# revision 54
# baseline (speedup 1.0000x reference)
"""Distributed causal multi-head attention for 8 TRN2 NeuronCores.

Problem: x[4,2048,1024], per-head Q/K/V [16,64,1024], O [1024,1024].
  q,k,v = per-head projections of x; scores = q@k^T (no 1/sqrt(d));
  causal softmax; z = attn@v; out = z @ O^T.

Sharding (head-parallel): core j owns heads {2j, 2j+1} for ALL batches.
Per core:
  - x/Wq/Wk in fp16 (10-bit mantissa): scores are ~N(0, 64) with no 1/sqrt(d)
    scaling, so exp() amplifies absolute score error; bf16 inputs would give
    ~4% output error while fp16 gives ~0.5% and runs at full PE rate
    (f32r runs at half rate; f32 at quarter rate).
  - scoresT [k, q] layout: the softmax denominator comes for free from a
    ones-column appended to the PV stationary operand (psum row 64 = l);
    exp runs on ACT from 2-bank psum groups, psum -> sbuf bf16.
  - causal mask applied post-exp via gpsimd.affine_select (multiplicative
    zeroing, exact vs the -1e10 reference mask).
  - z is normalized (reciprocal_approx_fast + gpsimd partition_broadcast)
    and exchanged via THREE 8-core AllToAlls: A2A_1 (1MB, macros {1,3} =
    each rank's second 512 output rows) fires after phase I and hides under
    phase II; the tail pair A2A_2a/2b (512KB each, macros {0,2}) pipelines
    with the O-projection. Core j ends with ALL 16 heads' z for its output
    rows (batch j//2, seq half j%2); O-projection is fully local.
  - phase I = all projections + attention {1,3} (PE-bound: proj has no exp
    work to hide), phase II = attention {0,2} (ACT/exp-bound, so O-proj A
    chunks interleave on the PE's slack). Within a unit, scores(g+1) is
    emitted before PV(g) so the in-order PE never waits on exp(g).
"""

import os

import numpy as np
import ml_dtypes

import concourse.mybir as mybir
import concourse.tile as tile
from concourse.tile import add_dep_helper
from concourse import bacc
from concourse.bass_utils import run_bass_kernel_spmd

BF16 = mybir.dt.bfloat16
F32 = mybir.dt.float32
F32R = mybir.dt.float32r
FP16 = mybir.dt.float16

B, M, NH, DH = 4, 1024, 16, 64
NCORES = 8

LAST_EXEC_TIME_NS = None


def build(S=2048):
    NM = S // 512          # 512-wide q-macros per batch
    GQ = B * S             # global q positions
    CH = GQ // NCORES      # AllToAll chunk width (= output rows per core)
    NQC = CH // 128        # output q-chunks per core

    nc = bacc.Bacc("TRN2", target_bir_lowering=False, debug=False, num_devices=NCORES)
    xt_ext = nc.dram_tensor("xt", [B, M, S], FP16, kind="ExternalInput")
    wqk_ext = nc.dram_tensor("wqk", [M, 256], FP16, kind="ExternalInput")
    wv_ext = nc.dram_tensor("wv", [M, 128], FP16, kind="ExternalInput")
    ot_ext = nc.dram_tensor("ot", [M, M], BF16, kind="ExternalInput")
    # fp16 output (~5e-4 rounding, well within budget) halves the tail
    # out-write traffic; kernel() casts back to f32.
    out_ext = nc.dram_tensor("out", [CH, M], FP16, kind="ExternalOutput")

    Exp = mybir.ActivationFunctionType.Exp

    with (
        tile.TileContext(nc) as tc,
        tc.tile_pool(name="wpool", bufs=1) as wpool,
        tc.tile_pool(name="xt", bufs=20) as xt_pool,
        tc.tile_pool(name="qk", bufs=1) as qk_pool,
        tc.tile_pool(name="kz", bufs=1) as kz_pool,
        tc.tile_pool(name="vp", bufs=1) as v_pool,
        tc.tile_pool(name="ep", bufs=6) as e_pool,
        tc.tile_pool(name="zp", bufs=12) as z_pool,
        tc.tile_pool(name="zr", bufs=1) as zr_pool,
        tc.tile_pool(name="ob", bufs=2) as ob_pool,
        tc.tile_pool(name="nrm", bufs=3) as nrm_pool,
        tc.tile_pool(name="ps_sc", bufs=2, space="PSUM") as ps_sc,
        tc.tile_pool(name="ps_z", bufs=1, space="PSUM") as ps_z,
        tc.tile_pool(name="ps_gen", bufs=2, space="PSUM") as ps_gen,
        tc.tile_pool(name="dram", bufs=1, space="DRAM") as dram,
    ):
        # ---- weights (resident) ----
        # x/weight loads NEVER touch the sync queue: sync is reserved for
        # z writes (softmax-dependent) so x prefetch can't stall behind them,
        # and z writes never wait behind bulk x traffic. Only SP (sync),
        # Activation (scalar) and Pool (gpsimd) can issue DMAs.
        xq = [nc.scalar, nc.gpsimd]
        wqk_sb, wv_sb, ot_sb = [], [], []
        for m in range(8):
            t = wpool.tile([128, 256], FP16, name=f"wqk{m}", tag=f"wqk{m}")
            xq[m % 2].dma_start(t[:], wqk_ext[128 * m:128 * (m + 1), :])
            wqk_sb.append(t)
            t = wpool.tile([128, 128], FP16, name=f"wv{m}", tag=f"wv{m}")
            nc.gpsimd.dma_start(t[:], wv_ext[128 * m:128 * (m + 1), :])
            wv_sb.append(t)
            t = wpool.tile([128, 1024], BF16, name=f"ot{m}", tag=f"ot{m}")
            ot_sb.append(t)

        CH2 = CH // 2
        CH4 = CH // 4
        # A2A_1 (hidden under phase II) stays one 1MB op; A2A_2 is split in
        # two 512KB ops so the tail pipelines: O-proj on the first half runs
        # while the second half is still on the CC cores.
        a2a_in1 = dram.tile([NCORES, 128, CH2], BF16, name="a2a_in1")
        a2a_out1 = dram.tile([NCORES, 128, CH2], BF16, name="a2a_out1")
        a2a_in2a = dram.tile([NCORES, 128, CH4], BF16, name="a2a_in2a")
        a2a_out2a = dram.tile([NCORES, 128, CH4], BF16, name="a2a_out2a")
        a2a_in2b = dram.tile([NCORES, 128, CH4], BF16, name="a2a_in2b")
        a2a_out2b = dram.tile([NCORES, 128, CH4], BF16, name="a2a_out2b")

        qk_sb = {}   # (ct, b, mq) -> [128, 512] fp16; ct0 = qT (2 heads), ct1 = kT
        kz_sb = {}   # (h, b, mk) -> [128, 512] fp16 zero-padded per-head kT
        v_sb = {}    # (b, k_tile) -> [128, 130] bf16: [vA(64) | 1 | vB(64) | 1]

        def emit_xt(b, mq, three_way=False):
            # during startup (before any z writes exist) sync is free, so
            # batches 0-1 of phase I also use it: 3-queue x feed for the
            # startup burst where the PE otherwise starves.
            qs = [nc.scalar, nc.gpsimd, nc.sync] if three_way else xq
            xts = []
            for m in range(8):
                t = xt_pool.tile([128, 512], FP16, name="xtc")
                qs[m % len(qs)].dma_start(
                    t[:], xt_ext[b, 128 * m:128 * (m + 1), 512 * mq:512 * (mq + 1)]
                )
                xts.append(t)
            return xts

        def emit_proj(b, mq, xts, fin=None):
            for ct in range(2):
                ps = ps_gen.tile([128, 512], F32, name="psqk", tag="gen")
                for m in range(8):
                    nc.tensor.matmul(
                        ps[:],
                        wqk_sb[m][:, 128 * ct:128 * (ct + 1)],
                        xts[m][:],
                        start=(m == 0),
                        stop=(m == 7),
                    )
                if ct == 0:
                    t = qk_pool.tile(
                        [128, 512], FP16, name=f"qk{ct}_{b}_{mq}", tag=f"qk{ct}_{b}_{mq}"
                    )
                    nc.vector.tensor_copy(t[:], ps[:])
                    qk_sb[(ct, b, mq)] = t
                    # the previous attention unit's deferred tail (final PV +
                    # normalize) lands here: the ct0 matmuls above fill the
                    # PE while that unit's last exp drains on ACT
                    if fin is not None:
                        fin()
                else:
                    # kT is consumed only as zero-padded per-head copies:
                    # K=128 scores matmuls run at full rate while 64-row
                    # quadrant matmuls measured ~1.5x slower per instruction.
                    # Copies on DVE: ACT is the attention pacer (exp), keep
                    # it exp-only.
                    for h in range(2):
                        kz = kz_pool.tile(
                            [128, 512], FP16, name=f"kz{h}_{b}_{mq}",
                            tag=f"kz{h}_{b}_{mq}",
                        )
                        nc.vector.memset(kz[64 - 64 * h:128 - 64 * h, :], 0.0)
                        nc.vector.tensor_copy(
                            kz[64 * h:64 * (h + 1), :],
                            ps[64 * h:64 * (h + 1), :],
                        )
                        kz_sb[(h, b, mq)] = kz
            for stl in range(4):
                ps = ps_gen.tile([128, 128], F32, name="psv", tag="gen")
                for m in range(8):
                    nc.tensor.matmul(
                        ps[:],
                        xts[m][:, 128 * stl:128 * (stl + 1)],
                        wv_sb[m][:],
                        start=(m == 0),
                        stop=(m == 7),
                    )
                kt = 4 * mq + stl
                # layout [vA(64) | 1 | vB(64) | 1]: ones column makes PV row 64
                # the softmax denominator; z dims land on rows 0..63 (DVE
                # partition ranges must start at 0/32/64/96, so z-rows-first)
                vt = v_pool.tile([128, 130], BF16, name=f"v_{b}_{kt}", tag=f"v_{b}_{kt}")
                nc.gpsimd.memset(vt[:, 64:65], 1.0)
                nc.gpsimd.memset(vt[:, 129:130], 1.0)
                nc.vector.tensor_copy(
                    vt[:].rearrange("p (g c) -> p g c", g=2)[:, :, 0:64],
                    ps[:].rearrange("p (g c) -> p g c", g=2),
                )
                v_sb[(b, kt)] = vt

        def emit_attn(b, mq, fin_prev=None, defer=False):
            nk = 4 * (mq + 1)
            # pz is allocated lazily at the first PV so pool-slot WAR
            # tracking stays consistent with deferred tails (the previous
            # unit's final PV may be emitted after this unit starts)
            pzc = []

            def get_pz():
                if not pzc:
                    pzc.append(ps_z.tile([128, 1024], F32, name="pz", tag="pz"))
                return pzc[0]

            def emit_scores_exp(g):
                # per head: 2 scores matmuls then immediately the exp, so ACT
                # starts on head 0 while the PE does head 1's scores. One
                # full-width exp per (group, head): ACT instructions have
                # ~700ns fixed overhead, so fewer/wider beats masked-region
                # skipping (measured +24us ACT when split per ktile).
                es = []
                for h in range(2):
                    psc = ps_sc.tile([128, 1024], F32, name="psc", tag="sc")
                    for kk in range(2):
                        kt = 2 * g + kk
                        mk, ktl = kt // 4, kt % 4
                        # diagonal tiles: q columns < 128*d are fully masked,
                        # so skip them in both scores and PV. The psum left
                        # unwritten holds stale-but-finite values whose exp
                        # is zeroed by the affine_select (q < 128d => masked).
                        d = kt - 4 * mq
                        q0 = 128 * d if d > 0 else 0
                        nc.tensor.matmul(
                            psc[:, 512 * kk + q0:512 * (kk + 1)],
                            kz_sb[(h, b, mk)][:, 128 * ktl:128 * (ktl + 1)],
                            qk_sb[(0, b, mq)][:, q0:512],
                            start=True,
                            stop=True,
                        )
                    e = e_pool.tile([128, 1024], BF16, name="etile")
                    nc.scalar.activation(e[:], psc[:], Exp)
                    es.append(e)
                for kk in range(2):
                    kt = 2 * g + kk
                    d = kt - 4 * mq
                    if d >= 0:  # diagonal: zero where k_local + 128*d > q_local
                        for h in range(2):
                            sl = es[h][:, 512 * kk:512 * (kk + 1)]
                            nc.gpsimd.affine_select(
                                out=sl,
                                in_=sl,
                                compare_op=mybir.AluOpType.is_ge,
                                fill=0.0,
                                base=-128 * d,
                                pattern=[[1, 512]],
                                channel_multiplier=-1,
                            )
                return es

            last_pe = [None]

            def emit_pv(g, es):
                pz = get_pz()
                for kk in range(2):
                    kt = 2 * g + kk
                    d = kt - 4 * mq
                    q0 = 128 * d if d > 0 else 0
                    vt = v_sb[(b, kt)]
                    for h in range(2):
                        last_pe[0] = nc.tensor.matmul(
                            pz[0:65, 512 * h + q0:512 * h + 512],
                            vt[:, 65 * h:65 * h + 65],
                            es[h][:, 512 * kk + q0:512 * (kk + 1)],
                            start=(kt == 0),
                            stop=(kt == nk - 1),
                        )

            # software pipeline: scores(g+1) is emitted BEFORE PV(g). The PE
            # queue is in-order, so otherwise PV(g)'s wait on exp(g) blocks
            # scores(g+1) that could already run - ~1us/group of PE idle.
            # The final PV + normalize are deferred (fin) so the NEXT
            # proj/unit's first matmuls can fill the last exp's latency; the
            # previous unit's fin lands right after this unit's first group.
            prev = None
            first = True
            for g in range(nk // 2):
                es = emit_scores_exp(g)
                if first:
                    if fin_prev is not None:
                        fin_prev()
                    first = False
                if prev is not None:
                    emit_pv(prev[0], prev[1])
                prev = (g, es)

            def fin():
                emit_tail(prev)

            def emit_tail(prev):
                emit_pv(prev[0], prev[1])
                pz = get_pz()
                # normalize: pz row 64 of each half = l. partition_broadcast
                # only reads from base partition 0, and DVE can't shift
                # partitions, so DMA the l row from psum partition 64 to
                # sbuf partition 0 first.
                zcp = nrm_pool.tile([65, 1024], F32, name="zcp", tag="zcp")
                nc.vector.tensor_copy(zcp[:, 0:512], pz[0:65, 0:512])
                nc.scalar.activation(
                    zcp[:, 512:1024],
                    pz[0:65, 512:1024],
                    mybir.ActivationFunctionType.Copy,
                )
                l0 = nrm_pool.tile([1, 1024], F32, name="l0", tag="l0")
                nc.gpsimd.dma_start(l0[:], zcp[64:65, :])
                rec = nrm_pool.tile([1, 1024], F32, name="rec", tag="rec")
                nc.vector.reciprocal_approx_fast(rec[:], l0[:])
                bc = nrm_pool.tile([64, 1024], F32, name="bc", tag="bc")
                nc.gpsimd.partition_broadcast(bc[:], rec[0:1, :])
                gcol = b * S + 512 * mq
                for h in range(2):
                    zt = z_pool.tile([64, 512], BF16, name="ztile")
                    nc.vector.tensor_mul(
                        zt[:],
                        zcp[0:64, 512 * h:512 * h + 512],
                        bc[:, 512 * h:512 * h + 512],
                    )
                    # routing: each rank's SECOND 512 rows (macros {1,3}) go
                    # to A2A_1 (fires after phase I); its first 512 rows
                    # ({0,2}) split across the tail collectives A2A_2a/2b.
                    w = 0
                    while w < 512:
                        gc = gcol + w
                        j, r = gc // CH, gc % CH
                        if r >= CH2:
                            buf, off = a2a_in1, r - CH2
                            ww = min(512 - w, CH - r)
                        elif r < CH4:
                            buf, off = a2a_in2a, r
                            ww = min(512 - w, CH4 - r)
                        else:
                            buf, off = a2a_in2b, r - CH4
                            ww = min(512 - w, CH2 - r)
                        nc.sync.dma_start(
                            buf[j, 64 * h:64 * (h + 1), off:off + ww],
                            zt[:, w:w + ww],
                        )
                        w += ww

            if defer:
                return fin, last_pe[0]
            fin()
            return None, last_pe[0]

        def emit_a2a(a2a_in, a2a_out):
            nc.gpsimd.collective_compute(
                "AllToAll",
                mybir.AluOpType.bypass,
                replica_groups=[list(range(NCORES))],
                ins=[a2a_in[:].opt()],
                outs=[a2a_out[:].opt()],
            )

        def emit_zrecv(a2a_out, tagp, width):
            # zrecv on sync: data-gated on the collective's completion
            # semaphore. Emitted as early as program order allows so the
            # reads stream during compute, not on the tail critical path.
            zrecv = []
            for ct in range(8):
                t = zr_pool.tile(
                    [128, width], BF16, name=f"zr{tagp}{ct}", tag=f"zr{tagp}{ct}"
                )
                nc.sync.dma_start(t[:], a2a_out[ct, :, :])
                zrecv.append(t)
            return zrecv

        def emit_oproj_qc(zrecv, row0, qc, after=None):
            # `after` pins the chunk's first matmul behind the intended
            # attention unit: the Tile scheduler otherwise hoists O-proj
            # (whose zrecv stationary load waits on a collective) ahead of
            # attention on the in-order PE queue, stalling it for the whole
            # collective (measured 49us).
            ob = ob_pool.tile([128, 1024], FP16, name="ob")
            pss = [
                ps_gen.tile([128, 512], F32, name="pso", tag="gen")
                for _ in range(2)
            ]
            first_mm = None
            for ct in range(8):
                for mh in range(2):
                    mm = nc.tensor.matmul(
                        pss[mh][:],
                        zrecv[ct][:, 128 * qc:128 * (qc + 1)],
                        ot_sb[ct][:, 512 * mh:512 * (mh + 1)],
                        start=(ct == 0),
                        stop=(ct == 7),
                    )
                    if first_mm is None:
                        first_mm = mm
                        if after is not None:
                            add_dep_helper(
                                mm.ins,
                                after.ins,
                                sync=False,
                                reason="keep O-proj chunk at its emission slot",
                            )
            for mh in range(2):
                nc.vector.tensor_copy(ob[:, 512 * mh:512 * (mh + 1)], pss[mh][:])
            r = row0 + 128 * qc
            nc.gpsimd.dma_start(out_ext[r:r + 128, :], ob[:])
            return first_mm

        # phase I: ALL projections + attention macros {1, 3} (the A2A_1 set:
        # each rank's second 512 output rows). {1,3} in phase I balances the
        # engines: proj (PE-only, no exp) interleaves with the exp-heavy big
        # macros, and it spreads the 16MB x stream over the whole phase.
        pend = {}
        pend[(0, 0)] = emit_xt(0, 0, three_way=True)
        pend[(0, 1)] = emit_xt(0, 1, three_way=True)
        fin = None
        for b in range(B):
            emit_proj(b, 0, pend.pop((b, 0)), fin=fin)
            fin = None
            emit_proj(b, 1, pend.pop((b, 1)))
            pend[(b, 2)] = emit_xt(b, 2)
            pend[(b, 3)] = emit_xt(b, 3)
            f1, _ = emit_attn(b, 1, defer=True)
            emit_proj(b, 2, pend.pop((b, 2)), fin=f1)
            emit_proj(b, 3, pend.pop((b, 3)))
            if b + 1 < B:
                pend[(b + 1, 0)] = emit_xt(b + 1, 0)
                pend[(b + 1, 1)] = emit_xt(b + 1, 1)
            fin, _ = emit_attn(b, 3, defer=True)
        fin()  # flush attn(3,3)'s tail: A2A_1 needs its z writes now
        emit_a2a(a2a_in1, a2a_out1)
        for m in range(8):
            nc.scalar.dma_start(ot_sb[m][:], ot_ext[128 * m:128 * (m + 1), :])
        # zrecv for O-proj A is emitted BEFORE phase II: on the sync queue it
        # blocks (waiting on A2A_1) ahead of the first phase-II z writes, but
        # the enlarged z pool absorbs that backlog, and the reads complete
        # early in phase II — off the tail critical path.
        zrecv_a = emit_zrecv(a2a_out1, "a", CH2)
        # phase II: macros {0, 2} - ACT-bound (all exp, little proj work), so
        # the PE has slack: O-proj A chunks are interleaved between the later
        # attention units (by when A2A_1 and zrecv_a have certainly landed),
        # and its last chunk fills the PE while A2A_2a drains.
        units2 = [(bb, mqq) for bb in range(B) for mqq in (0, 2)]
        link = None
        finII = None
        for i, (bb, mqq) in enumerate(units2):
            finII, lpe = emit_attn(bb, mqq, fin_prev=finII, defer=True)
            if i >= 5:
                link = emit_oproj_qc(zrecv_a, CH2, i - 5, after=lpe)
        # O-proj A's last chunk is the PE filler for the final unit's exp
        # latency; the A2A_2 triggers follow the flushed tail's z writes and
        # are emitted before the chunk's gpsimd out-write issue
        link = emit_oproj_qc(zrecv_a, CH2, 3, after=link)
        finII()
        emit_a2a(a2a_in2a, a2a_out2a)
        emit_a2a(a2a_in2b, a2a_out2b)
        zrecv_b1 = emit_zrecv(a2a_out2a, "b1", CH4)
        zrecv_b2 = emit_zrecv(a2a_out2b, "b2", CH4)
        for qc in range(2):
            link = emit_oproj_qc(zrecv_b1, 0, qc, after=link)
        for qc in range(2):
            link = emit_oproj_qc(zrecv_b2, CH4, qc, after=link)

    nc.compile()
    return nc


_BUILT = {}


def _get_built(S):
    if S not in _BUILT:
        _BUILT[S] = build(S)
    return _BUILT[S]


def prep_inputs(x, Q, K, V, O):
    x = np.asarray(x, dtype=np.float32)
    Q = np.asarray(Q, dtype=np.float32)
    K = np.asarray(K, dtype=np.float32)
    V = np.asarray(V, dtype=np.float32)
    O = np.asarray(O, dtype=np.float32)
    xt = np.ascontiguousarray(np.transpose(x, (0, 2, 1))).astype(np.float16)  # [B, M, S]
    ot = np.ascontiguousarray(O.T).astype(ml_dtypes.bfloat16)  # [a, m], a = n*64+h
    in_maps = []
    for j in range(NCORES):
        hA, hB = 2 * j, 2 * j + 1
        wqk = np.ascontiguousarray(
            np.concatenate([Q[hA], Q[hB], K[hA], K[hB]], axis=0).T
        ).astype(np.float16)  # [1024, 256]
        wv = np.ascontiguousarray(
            np.concatenate([V[hA], V[hB]], axis=0).T
        ).astype(np.float16)  # [1024, 128]
        in_maps.append({"xt": xt, "wqk": wqk, "wv": wv, "ot": ot})
    return in_maps


def kernel(x, Q, K, V, O):
    global LAST_EXEC_TIME_NS
    x = np.asarray(x)
    S = x.shape[1]
    nc = _get_built(S)
    in_maps = prep_inputs(x, Q, K, V, O)
    trace = bool(int(os.environ.get("ATTN_TRACE", "0")))
    res = run_bass_kernel_spmd(nc, in_maps, list(range(NCORES)), trace=trace)
    LAST_EXEC_TIME_NS = res.exec_time_ns
    out = np.zeros((B, S, M), np.float32)
    half = S // 2
    for j in range(NCORES):
        b, hh = j // 2, j % 2
        out[b, hh * half:(hh + 1) * half, :] = res.results[j]["out"]
    return out



# revision 55
# speedup vs baseline: 1.0135x; 1.0135x over previous
"""Distributed causal multi-head attention for 8 TRN2 NeuronCores.

Problem: x[4,2048,1024], per-head Q/K/V [16,64,1024], O [1024,1024].
  q,k,v = per-head projections of x; scores = q@k^T (no 1/sqrt(d));
  causal softmax; z = attn@v; out = z @ O^T.

Sharding (head-parallel): core j owns heads {2j, 2j+1} for ALL batches.
Per core:
  - x/Wq/Wk in fp16 (10-bit mantissa): scores are ~N(0, 64) with no 1/sqrt(d)
    scaling, so exp() amplifies absolute score error; bf16 inputs would give
    ~4% output error while fp16 gives ~0.5% and runs at full PE rate
    (f32r runs at half rate; f32 at quarter rate).
  - scoresT [k, q] layout: the softmax denominator comes for free from a
    ones-column appended to the PV stationary operand (psum row 64 = l);
    exp runs on ACT from 2-bank psum groups, psum -> sbuf bf16.
  - causal mask applied post-exp via gpsimd.affine_select (multiplicative
    zeroing, exact vs the -1e10 reference mask).
  - z is normalized (reciprocal_approx_fast + gpsimd partition_broadcast)
    and exchanged via THREE 8-core AllToAlls: A2A_1 (1MB, macros {1,3} =
    each rank's second 512 output rows) fires after phase I and hides under
    phase II; the tail pair A2A_2a/2b (512KB each, macros {0,2}) pipelines
    with the O-projection. Core j ends with ALL 16 heads' z for its output
    rows (batch j//2, seq half j%2); O-projection is fully local.
  - phase I = all projections + attention {1,3} (PE-bound: proj has no exp
    work to hide), phase II = attention {0,2} (ACT/exp-bound, so O-proj A
    chunks interleave on the PE's slack). Within a unit, scores(g+1) is
    emitted before PV(g) so the in-order PE never waits on exp(g), and each
    unit's final PV + normalize are deferred into the next proj/unit.
  - measured hazards baked in: ACT ops cost ~700ns fixed (never split exps);
    sub-128-row quadrant matmuls run ~1.5x slower (keep kz zero-padded);
    the Tile scheduler hoists O-proj's zrecv weight loads ahead of attention
    unless pinned (add_dep_helper anchors); x/weights and softmax-dependent
    z writes must live on disjoint DMA queues.
"""

import os

import numpy as np
import ml_dtypes

import concourse.mybir as mybir
import concourse.tile as tile
from concourse.tile import add_dep_helper
from concourse import bacc
from concourse.bass_utils import run_bass_kernel_spmd

BF16 = mybir.dt.bfloat16
F32 = mybir.dt.float32
F32R = mybir.dt.float32r
FP16 = mybir.dt.float16

B, M, NH, DH = 4, 1024, 16, 64
NCORES = 8

LAST_EXEC_TIME_NS = None


def build(S=2048):
    NM = S // 512          # 512-wide q-macros per batch
    GQ = B * S             # global q positions
    CH = GQ // NCORES      # AllToAll chunk width (= output rows per core)
    NQC = CH // 128        # output q-chunks per core

    nc = bacc.Bacc("TRN2", target_bir_lowering=False, debug=False, num_devices=NCORES)
    xt_ext = nc.dram_tensor("xt", [B, M, S], FP16, kind="ExternalInput")
    wqk_ext = nc.dram_tensor("wqk", [M, 256], FP16, kind="ExternalInput")
    wv_ext = nc.dram_tensor("wv", [M, 128], FP16, kind="ExternalInput")
    ot_ext = nc.dram_tensor("ot", [M, M], BF16, kind="ExternalInput")
    # fp16 output (~5e-4 rounding, well within budget) halves the tail
    # out-write traffic; kernel() casts back to f32.
    out_ext = nc.dram_tensor("out", [CH, M], FP16, kind="ExternalOutput")

    Exp = mybir.ActivationFunctionType.Exp

    with (
        tile.TileContext(nc) as tc,
        tc.tile_pool(name="wpool", bufs=1) as wpool,
        tc.tile_pool(name="xt", bufs=20) as xt_pool,
        tc.tile_pool(name="qk", bufs=1) as qk_pool,
        tc.tile_pool(name="kz", bufs=1) as kz_pool,
        tc.tile_pool(name="vp", bufs=1) as v_pool,
        tc.tile_pool(name="ep", bufs=6) as e_pool,
        tc.tile_pool(name="zp", bufs=12) as z_pool,
        tc.tile_pool(name="zr", bufs=1) as zr_pool,
        tc.tile_pool(name="ob", bufs=2) as ob_pool,
        tc.tile_pool(name="nrm", bufs=3) as nrm_pool,
        tc.tile_pool(name="ps_sc", bufs=2, space="PSUM") as ps_sc,
        tc.tile_pool(name="ps_z", bufs=1, space="PSUM") as ps_z,
        tc.tile_pool(name="ps_gen", bufs=2, space="PSUM") as ps_gen,
        tc.tile_pool(name="dram", bufs=1, space="DRAM") as dram,
    ):
        # ---- weights (resident) ----
        # x/weight loads NEVER touch the sync queue: sync is reserved for
        # z writes (softmax-dependent) so x prefetch can't stall behind them,
        # and z writes never wait behind bulk x traffic. Only SP (sync),
        # Activation (scalar) and Pool (gpsimd) can issue DMAs.
        xq = [nc.scalar, nc.gpsimd]
        wqk_sb, wv_sb, ot_sb = [], [], []
        for m in range(8):
            t = wpool.tile([128, 256], FP16, name=f"wqk{m}", tag=f"wqk{m}")
            xq[m % 2].dma_start(t[:], wqk_ext[128 * m:128 * (m + 1), :])
            wqk_sb.append(t)
            t = wpool.tile([128, 128], FP16, name=f"wv{m}", tag=f"wv{m}")
            nc.gpsimd.dma_start(t[:], wv_ext[128 * m:128 * (m + 1), :])
            wv_sb.append(t)
            t = wpool.tile([128, 1024], BF16, name=f"ot{m}", tag=f"ot{m}")
            ot_sb.append(t)

        CH2 = CH // 2
        CH4 = CH // 4
        # A2A_1 (hidden under phase II) stays one 1MB op; A2A_2 is split in
        # two 512KB ops so the tail pipelines: O-proj on the first half runs
        # while the second half is still on the CC cores.
        a2a_in1 = dram.tile([NCORES, 128, CH2], BF16, name="a2a_in1")
        a2a_out1 = dram.tile([NCORES, 128, CH2], BF16, name="a2a_out1")
        a2a_in2a = dram.tile([NCORES, 128, CH4], BF16, name="a2a_in2a")
        a2a_out2a = dram.tile([NCORES, 128, CH4], BF16, name="a2a_out2a")
        a2a_in2b = dram.tile([NCORES, 128, CH4], BF16, name="a2a_in2b")
        a2a_out2b = dram.tile([NCORES, 128, CH4], BF16, name="a2a_out2b")

        qk_sb = {}   # (ct, b, mq) -> [128, 512] fp16; ct0 = qT (2 heads), ct1 = kT
        kz_sb = {}   # (h, b, mk) -> [128, 512] fp16 zero-padded per-head kT
        v_sb = {}    # (b, k_tile) -> [128, 130] bf16: [vA(64) | 1 | vB(64) | 1]

        def emit_xt(b, mq, three_way=False):
            # during startup (before any z writes exist) sync is free, so
            # batches 0-1 of phase I also use it: 3-queue x feed for the
            # startup burst where the PE otherwise starves.
            qs = [nc.scalar, nc.gpsimd, nc.sync] if three_way else xq
            xts = []
            for m in range(8):
                t = xt_pool.tile([128, 512], FP16, name="xtc")
                qs[m % len(qs)].dma_start(
                    t[:], xt_ext[b, 128 * m:128 * (m + 1), 512 * mq:512 * (mq + 1)]
                )
                xts.append(t)
            return xts

        def emit_proj(b, mq, xts, fin=None):
            for ct in range(2):
                ps = ps_gen.tile([128, 512], F32, name="psqk", tag="gen")
                for m in range(8):
                    nc.tensor.matmul(
                        ps[:],
                        wqk_sb[m][:, 128 * ct:128 * (ct + 1)],
                        xts[m][:],
                        start=(m == 0),
                        stop=(m == 7),
                    )
                if ct == 0:
                    t = qk_pool.tile(
                        [128, 512], FP16, name=f"qk{ct}_{b}_{mq}", tag=f"qk{ct}_{b}_{mq}"
                    )
                    nc.vector.tensor_copy(t[:], ps[:])
                    qk_sb[(ct, b, mq)] = t
                    # the previous attention unit's deferred tail (final PV +
                    # normalize) lands here: the ct0 matmuls above fill the
                    # PE while that unit's last exp drains on ACT
                    if fin is not None:
                        fin()
                else:
                    # kT is consumed only as zero-padded per-head copies:
                    # K=128 scores matmuls run at full rate while 64-row
                    # quadrant matmuls measured ~1.5x slower per instruction.
                    # Copies on DVE: ACT is the attention pacer (exp), keep
                    # it exp-only.
                    for h in range(2):
                        kz = kz_pool.tile(
                            [128, 512], FP16, name=f"kz{h}_{b}_{mq}",
                            tag=f"kz{h}_{b}_{mq}",
                        )
                        nc.vector.memset(kz[64 - 64 * h:128 - 64 * h, :], 0.0)
                        nc.vector.tensor_copy(
                            kz[64 * h:64 * (h + 1), :],
                            ps[64 * h:64 * (h + 1), :],
                        )
                        kz_sb[(h, b, mq)] = kz
            for stl in range(4):
                ps = ps_gen.tile([128, 128], F32, name="psv", tag="gen")
                for m in range(8):
                    nc.tensor.matmul(
                        ps[:],
                        xts[m][:, 128 * stl:128 * (stl + 1)],
                        wv_sb[m][:],
                        start=(m == 0),
                        stop=(m == 7),
                    )
                kt = 4 * mq + stl
                # layout [vA(64) | 1 | vB(64) | 1]: ones column makes PV row 64
                # the softmax denominator; z dims land on rows 0..63 (DVE
                # partition ranges must start at 0/32/64/96, so z-rows-first)
                vt = v_pool.tile([128, 130], BF16, name=f"v_{b}_{kt}", tag=f"v_{b}_{kt}")
                nc.gpsimd.memset(vt[:, 64:65], 1.0)
                nc.gpsimd.memset(vt[:, 129:130], 1.0)
                nc.vector.tensor_copy(
                    vt[:].rearrange("p (g c) -> p g c", g=2)[:, :, 0:64],
                    ps[:].rearrange("p (g c) -> p g c", g=2),
                )
                v_sb[(b, kt)] = vt

        def emit_attn(b, mq, fin_prev=None, defer=False):
            nk = 4 * (mq + 1)
            # pz is allocated lazily at the first PV so pool-slot WAR
            # tracking stays consistent with deferred tails (the previous
            # unit's final PV may be emitted after this unit starts)
            pzc = []

            def get_pz():
                if not pzc:
                    pzc.append(ps_z.tile([128, 1024], F32, name="pz", tag="pz"))
                return pzc[0]

            def emit_scores_exp(g):
                # per head: 2 scores matmuls then immediately the exp, so ACT
                # starts on head 0 while the PE does head 1's scores. One
                # full-width exp per (group, head): ACT instructions have
                # ~700ns fixed overhead, so fewer/wider beats masked-region
                # skipping (measured +24us ACT when split per ktile).
                es = []
                for h in range(2):
                    psc = ps_sc.tile([128, 1024], F32, name="psc", tag="sc")
                    for kk in range(2):
                        kt = 2 * g + kk
                        mk, ktl = kt // 4, kt % 4
                        # diagonal tiles: q columns < 128*d are fully masked,
                        # so skip them in both scores and PV. The psum left
                        # unwritten holds stale-but-finite values whose exp
                        # is zeroed by the affine_select (q < 128d => masked).
                        d = kt - 4 * mq
                        q0 = 128 * d if d > 0 else 0
                        nc.tensor.matmul(
                            psc[:, 512 * kk + q0:512 * (kk + 1)],
                            kz_sb[(h, b, mk)][:, 128 * ktl:128 * (ktl + 1)],
                            qk_sb[(0, b, mq)][:, q0:512],
                            start=True,
                            stop=True,
                        )
                    e = e_pool.tile([128, 1024], BF16, name="etile")
                    nc.scalar.activation(e[:], psc[:], Exp)
                    es.append(e)
                for kk in range(2):
                    kt = 2 * g + kk
                    d = kt - 4 * mq
                    if d >= 0:  # diagonal: zero where k_local + 128*d > q_local
                        for h in range(2):
                            sl = es[h][:, 512 * kk:512 * (kk + 1)]
                            nc.gpsimd.affine_select(
                                out=sl,
                                in_=sl,
                                compare_op=mybir.AluOpType.is_ge,
                                fill=0.0,
                                base=-128 * d,
                                pattern=[[1, 512]],
                                channel_multiplier=-1,
                            )
                return es

            last_pe = [None]

            def emit_pv(g, es):
                pz = get_pz()
                for kk in range(2):
                    kt = 2 * g + kk
                    d = kt - 4 * mq
                    q0 = 128 * d if d > 0 else 0
                    vt = v_sb[(b, kt)]
                    for h in range(2):
                        last_pe[0] = nc.tensor.matmul(
                            pz[0:65, 512 * h + q0:512 * h + 512],
                            vt[:, 65 * h:65 * h + 65],
                            es[h][:, 512 * kk + q0:512 * (kk + 1)],
                            start=(kt == 0),
                            stop=(kt == nk - 1),
                        )

            # software pipeline: scores(g+1) is emitted BEFORE PV(g). The PE
            # queue is in-order, so otherwise PV(g)'s wait on exp(g) blocks
            # scores(g+1) that could already run - ~1us/group of PE idle.
            # The final PV + normalize are deferred (fin) so the NEXT
            # proj/unit's first matmuls can fill the last exp's latency; the
            # previous unit's fin lands right after this unit's first group.
            prev = None
            first = True
            for g in range(nk // 2):
                es = emit_scores_exp(g)
                if first:
                    if fin_prev is not None:
                        fin_prev()
                    first = False
                if prev is not None:
                    emit_pv(prev[0], prev[1])
                prev = (g, es)

            def fin():
                emit_tail(prev)

            def emit_tail(prev):
                emit_pv(prev[0], prev[1])
                pz = get_pz()
                # normalize: pz row 64 of each half = l. partition_broadcast
                # only reads from base partition 0, and DVE can't shift
                # partitions, so DMA the l row from psum partition 64 to
                # sbuf partition 0 first.
                zcp = nrm_pool.tile([65, 1024], F32, name="zcp", tag="zcp")
                nc.vector.tensor_copy(zcp[:, 0:512], pz[0:65, 0:512])
                nc.scalar.activation(
                    zcp[:, 512:1024],
                    pz[0:65, 512:1024],
                    mybir.ActivationFunctionType.Copy,
                )
                l0 = nrm_pool.tile([1, 1024], F32, name="l0", tag="l0")
                nc.gpsimd.dma_start(l0[:], zcp[64:65, :])
                rec = nrm_pool.tile([1, 1024], F32, name="rec", tag="rec")
                nc.vector.reciprocal_approx_fast(rec[:], l0[:])
                bc = nrm_pool.tile([64, 1024], F32, name="bc", tag="bc")
                nc.gpsimd.partition_broadcast(bc[:], rec[0:1, :])
                gcol = b * S + 512 * mq
                for h in range(2):
                    zt = z_pool.tile([64, 512], BF16, name="ztile")
                    nc.vector.tensor_mul(
                        zt[:],
                        zcp[0:64, 512 * h:512 * h + 512],
                        bc[:, 512 * h:512 * h + 512],
                    )
                    # routing: each rank's SECOND 512 rows (macros {1,3}) go
                    # to A2A_1 (fires after phase I); its first 512 rows
                    # ({0,2}) split across the tail collectives A2A_2a/2b.
                    w = 0
                    while w < 512:
                        gc = gcol + w
                        j, r = gc // CH, gc % CH
                        if r >= CH2:
                            buf, off = a2a_in1, r - CH2
                            ww = min(512 - w, CH - r)
                        elif r < CH4:
                            buf, off = a2a_in2a, r
                            ww = min(512 - w, CH4 - r)
                        else:
                            buf, off = a2a_in2b, r - CH4
                            ww = min(512 - w, CH2 - r)
                        nc.sync.dma_start(
                            buf[j, 64 * h:64 * (h + 1), off:off + ww],
                            zt[:, w:w + ww],
                        )
                        w += ww

            if defer:
                return fin, last_pe[0]
            fin()
            return None, last_pe[0]

        def emit_a2a(a2a_in, a2a_out):
            nc.gpsimd.collective_compute(
                "AllToAll",
                mybir.AluOpType.bypass,
                replica_groups=[list(range(NCORES))],
                ins=[a2a_in[:].opt()],
                outs=[a2a_out[:].opt()],
            )

        def emit_zrecv(a2a_out, tagp, width):
            # zrecv on sync: data-gated on the collective's completion
            # semaphore. Emitted as early as program order allows so the
            # reads stream during compute, not on the tail critical path.
            zrecv = []
            for ct in range(8):
                t = zr_pool.tile(
                    [128, width], BF16, name=f"zr{tagp}{ct}", tag=f"zr{tagp}{ct}"
                )
                nc.sync.dma_start(t[:], a2a_out[ct, :, :])
                zrecv.append(t)
            return zrecv

        def emit_oproj_qc(zrecv, row0, qc, after=None):
            # `after` pins the chunk's first matmul behind the intended
            # attention unit: the Tile scheduler otherwise hoists O-proj
            # (whose zrecv stationary load waits on a collective) ahead of
            # attention on the in-order PE queue, stalling it for the whole
            # collective (measured 49us).
            ob = ob_pool.tile([128, 1024], FP16, name="ob")
            pss = [
                ps_gen.tile([128, 512], F32, name="pso", tag="gen")
                for _ in range(2)
            ]
            first_mm = None
            for ct in range(8):
                for mh in range(2):
                    mm = nc.tensor.matmul(
                        pss[mh][:],
                        zrecv[ct][:, 128 * qc:128 * (qc + 1)],
                        ot_sb[ct][:, 512 * mh:512 * (mh + 1)],
                        start=(ct == 0),
                        stop=(ct == 7),
                    )
                    if first_mm is None:
                        first_mm = mm
                        if after is not None:
                            add_dep_helper(
                                mm.ins,
                                after.ins,
                                sync=False,
                                reason="keep O-proj chunk at its emission slot",
                            )
            for mh in range(2):
                nc.vector.tensor_copy(ob[:, 512 * mh:512 * (mh + 1)], pss[mh][:])
            r = row0 + 128 * qc
            nc.gpsimd.dma_start(out_ext[r:r + 128, :], ob[:])
            return first_mm

        # phase I: ALL projections + attention macros {1, 3} (the A2A_1 set:
        # each rank's second 512 output rows). {1,3} in phase I balances the
        # engines: proj (PE-only, no exp) interleaves with the exp-heavy big
        # macros, and it spreads the 16MB x stream over the whole phase.
        pend = {}
        pend[(0, 0)] = emit_xt(0, 0, three_way=True)
        pend[(0, 1)] = emit_xt(0, 1, three_way=True)
        fin = None
        for b in range(B):
            emit_proj(b, 0, pend.pop((b, 0)), fin=fin)
            fin = None
            emit_proj(b, 1, pend.pop((b, 1)))
            pend[(b, 2)] = emit_xt(b, 2)
            pend[(b, 3)] = emit_xt(b, 3)
            f1, _ = emit_attn(b, 1, defer=True)
            emit_proj(b, 2, pend.pop((b, 2)), fin=f1)
            emit_proj(b, 3, pend.pop((b, 3)))
            if b + 1 < B:
                pend[(b + 1, 0)] = emit_xt(b + 1, 0)
                pend[(b + 1, 1)] = emit_xt(b + 1, 1)
            fin, _ = emit_attn(b, 3, defer=True)
        fin()  # flush attn(3,3)'s tail: A2A_1 needs its z writes now
        emit_a2a(a2a_in1, a2a_out1)
        for m in range(8):
            nc.scalar.dma_start(ot_sb[m][:], ot_ext[128 * m:128 * (m + 1), :])
        # zrecv for O-proj A is emitted BEFORE phase II: on the sync queue it
        # blocks (waiting on A2A_1) ahead of the first phase-II z writes, but
        # the enlarged z pool absorbs that backlog, and the reads complete
        # early in phase II — off the tail critical path.
        zrecv_a = emit_zrecv(a2a_out1, "a", CH2)
        # phase II: macros {0, 2} - ACT-bound (all exp, little proj work), so
        # the PE has slack: O-proj A chunks are interleaved between the later
        # attention units (by when A2A_1 and zrecv_a have certainly landed),
        # and its last chunk fills the PE while A2A_2a drains.
        units2 = [(bb, mqq) for bb in range(B) for mqq in (0, 2)]
        link = None
        finII = None
        for i, (bb, mqq) in enumerate(units2):
            finII, lpe = emit_attn(bb, mqq, fin_prev=finII, defer=True)
            if i >= 5:
                link = emit_oproj_qc(zrecv_a, CH2, i - 5, after=lpe)
        # O-proj A's last chunk is the PE filler for the final unit's exp
        # latency; the A2A_2 triggers follow the flushed tail's z writes and
        # are emitted before the chunk's gpsimd out-write issue
        link = emit_oproj_qc(zrecv_a, CH2, 3, after=link)
        finII()
        emit_a2a(a2a_in2a, a2a_out2a)
        emit_a2a(a2a_in2b, a2a_out2b)
        zrecv_b1 = emit_zrecv(a2a_out2a, "b1", CH4)
        zrecv_b2 = emit_zrecv(a2a_out2b, "b2", CH4)
        for qc in range(2):
            link = emit_oproj_qc(zrecv_b1, 0, qc, after=link)
        for qc in range(2):
            link = emit_oproj_qc(zrecv_b2, CH4, qc, after=link)

    nc.compile()
    return nc


_BUILT = {}


def _get_built(S):
    if S not in _BUILT:
        _BUILT[S] = build(S)
    return _BUILT[S]


def prep_inputs(x, Q, K, V, O):
    x = np.asarray(x, dtype=np.float32)
    Q = np.asarray(Q, dtype=np.float32)
    K = np.asarray(K, dtype=np.float32)
    V = np.asarray(V, dtype=np.float32)
    O = np.asarray(O, dtype=np.float32)
    xt = np.ascontiguousarray(np.transpose(x, (0, 2, 1))).astype(np.float16)  # [B, M, S]
    ot = np.ascontiguousarray(O.T).astype(ml_dtypes.bfloat16)  # [a, m], a = n*64+h
    in_maps = []
    for j in range(NCORES):
        hA, hB = 2 * j, 2 * j + 1
        wqk = np.ascontiguousarray(
            np.concatenate([Q[hA], Q[hB], K[hA], K[hB]], axis=0).T
        ).astype(np.float16)  # [1024, 256]
        wv = np.ascontiguousarray(
            np.concatenate([V[hA], V[hB]], axis=0).T
        ).astype(np.float16)  # [1024, 128]
        in_maps.append({"xt": xt, "wqk": wqk, "wv": wv, "ot": ot})
    return in_maps


def kernel(x, Q, K, V, O):
    global LAST_EXEC_TIME_NS
    x = np.asarray(x)
    S = x.shape[1]
    nc = _get_built(S)
    in_maps = prep_inputs(x, Q, K, V, O)
    trace = bool(int(os.environ.get("ATTN_TRACE", "0")))
    res = run_bass_kernel_spmd(nc, in_maps, list(range(NCORES)), trace=trace)
    LAST_EXEC_TIME_NS = res.exec_time_ns
    out = np.zeros((B, S, M), np.float32)
    half = S // 2
    for j in range(NCORES):
        b, hh = j // 2, j % 2
        out[b, hh * half:(hh + 1) * half, :] = res.results[j]["out"]
    return out



# revision 58
# speedup vs baseline: 1.0558x; 1.0418x over previous
"""Distributed causal multi-head attention for 8 TRN2 NeuronCores.

Problem: x[4,2048,1024], per-head Q/K/V [16,64,1024], O [1024,1024].
  q,k,v = per-head projections of x; scores = q@k^T (no 1/sqrt(d));
  causal softmax; z = attn@v; out = z @ O^T.

Sharding (head-parallel): core j owns heads {2j, 2j+1} for ALL batches.
Per core:
  - x/Wq/Wk in fp16 (10-bit mantissa): scores are ~N(0, 64) with no 1/sqrt(d)
    scaling, so exp() amplifies absolute score error; bf16 inputs would give
    ~4% output error while fp16 gives ~0.5% and runs at full PE rate
    (f32r runs at half rate; f32 at quarter rate).
  - scoresT [k, q] layout: the softmax denominator comes for free from a
    ones-column appended to the PV stationary operand (psum row 64 = l);
    exp runs on ACT from 2-bank psum groups, psum -> sbuf bf16.
  - causal mask applied post-exp via gpsimd.affine_select (multiplicative
    zeroing, exact vs the -1e10 reference mask).
  - z is normalized (reciprocal_approx_fast + gpsimd partition_broadcast)
    and exchanged via THREE 8-core AllToAlls: A2A_1 (1MB, macros {1,3} =
    each rank's second 512 output rows) fires after phase I and hides under
    phase II; the tail pair A2A_2a/2b (512KB each, macros {0,2}) pipelines
    with the O-projection. Core j ends with ALL 16 heads' z for its output
    rows (batch j//2, seq half j%2); O-projection is fully local.
  - phase I = all projections + attention {1,3} (PE-bound: proj has no exp
    work to hide), phase II = attention {0,2} (ACT/exp-bound, so O-proj A
    chunks interleave on the PE's slack). Within a unit, scores(g+1) is
    emitted before PV(g) so the in-order PE never waits on exp(g), and each
    unit's final PV + normalize are deferred into the next proj/unit.
  - measured hazards baked in: ACT ops cost ~700ns fixed (never split exps);
    sub-128-row quadrant matmuls run ~1.5x slower (keep kz zero-padded);
    the Tile scheduler hoists O-proj's zrecv weight loads ahead of attention
    unless pinned (add_dep_helper anchors); x/weights and softmax-dependent
    z writes must live on disjoint DMA queues.
"""

import os

import numpy as np
import ml_dtypes

import concourse.mybir as mybir
import concourse.tile as tile
from concourse.tile import add_dep_helper
from concourse import bacc
from concourse.bass_utils import run_bass_kernel_spmd

BF16 = mybir.dt.bfloat16
F32 = mybir.dt.float32
F32R = mybir.dt.float32r
FP16 = mybir.dt.float16

B, M, NH, DH = 4, 1024, 16, 64
NCORES = 8

LAST_EXEC_TIME_NS = None


def build(S=2048):
    NM = S // 512          # 512-wide q-macros per batch
    GQ = B * S             # global q positions
    CH = GQ // NCORES      # AllToAll chunk width (= output rows per core)
    NQC = CH // 128        # output q-chunks per core

    nc = bacc.Bacc("TRN2", target_bir_lowering=False, debug=False, num_devices=NCORES)
    xt_ext = nc.dram_tensor("xt", [B, M, S], FP16, kind="ExternalInput")
    wqk_ext = nc.dram_tensor("wqk", [M, 256], FP16, kind="ExternalInput")
    wv_ext = nc.dram_tensor("wv", [M, 128], FP16, kind="ExternalInput")
    ot_ext = nc.dram_tensor("ot", [M, M], BF16, kind="ExternalInput")
    # fp16 output (~5e-4 rounding, well within budget) halves the tail
    # out-write traffic; kernel() casts back to f32.
    out_ext = nc.dram_tensor("out", [CH, M], FP16, kind="ExternalOutput")

    Exp = mybir.ActivationFunctionType.Exp

    with (
        tile.TileContext(nc) as tc,
        tc.tile_pool(name="wpool", bufs=1) as wpool,
        tc.tile_pool(name="xt", bufs=20) as xt_pool,
        tc.tile_pool(name="qk", bufs=1) as qk_pool,
        tc.tile_pool(name="kz", bufs=1) as kz_pool,
        tc.tile_pool(name="vp", bufs=1) as v_pool,
        tc.tile_pool(name="ep", bufs=6) as e_pool,
        tc.tile_pool(name="zp", bufs=12) as z_pool,
        tc.tile_pool(name="zr", bufs=1) as zr_pool,
        tc.tile_pool(name="ob", bufs=2) as ob_pool,
        tc.tile_pool(name="nrm", bufs=3) as nrm_pool,
        tc.tile_pool(name="ps_sc", bufs=2, space="PSUM") as ps_sc,
        tc.tile_pool(name="ps_z", bufs=1, space="PSUM") as ps_z,
        tc.tile_pool(name="ps_gen", bufs=2, space="PSUM") as ps_gen,
        tc.tile_pool(name="dram", bufs=1, space="DRAM") as dram,
    ):
        # ---- weights (resident) ----
        # x/weight loads NEVER touch the sync queue: sync is reserved for
        # z writes (softmax-dependent) so x prefetch can't stall behind them,
        # and z writes never wait behind bulk x traffic. Only SP (sync),
        # Activation (scalar) and Pool (gpsimd) can issue DMAs.
        xq = [nc.scalar, nc.gpsimd]
        wqk_sb, wv_sb, ot_sb = [], [], []
        for m in range(8):
            t = wpool.tile([128, 256], FP16, name=f"wqk{m}", tag=f"wqk{m}")
            xq[m % 2].dma_start(t[:], wqk_ext[128 * m:128 * (m + 1), :])
            wqk_sb.append(t)
            t = wpool.tile([128, 128], FP16, name=f"wv{m}", tag=f"wv{m}")
            nc.gpsimd.dma_start(t[:], wv_ext[128 * m:128 * (m + 1), :])
            wv_sb.append(t)
            t = wpool.tile([128, 1024], BF16, name=f"ot{m}", tag=f"ot{m}")
            ot_sb.append(t)

        CH2 = CH // 2
        CH4 = CH // 4
        # A2A_1 (hidden under phase II) stays one 1MB op; A2A_2 is split in
        # two 512KB ops so the tail pipelines: O-proj on the first half runs
        # while the second half is still on the CC cores.
        a2a_in1 = dram.tile([NCORES, 128, CH2], BF16, name="a2a_in1")
        a2a_out1 = dram.tile([NCORES, 128, CH2], BF16, name="a2a_out1")
        a2a_in2a = dram.tile([NCORES, 128, CH4], BF16, name="a2a_in2a")
        a2a_out2a = dram.tile([NCORES, 128, CH4], BF16, name="a2a_out2a")
        a2a_in2b = dram.tile([NCORES, 128, CH4], BF16, name="a2a_in2b")
        a2a_out2b = dram.tile([NCORES, 128, CH4], BF16, name="a2a_out2b")

        qk_sb = {}   # (ct, b, mq) -> [128, 512] fp16; ct0 = qT (2 heads), ct1 = kT
        kz_sb = {}   # (h, b, mk) -> [128, 512] fp16 zero-padded per-head kT
        v_sb = {}    # (b, k_tile) -> [128, 130] bf16: [vA(64) | 1 | vB(64) | 1]

        def emit_xt(b, mq, three_way=False):
            # during startup (before any z writes exist) sync is free, so
            # batches 0-1 of phase I also use it: 3-queue x feed for the
            # startup burst where the PE otherwise starves.
            qs = [nc.scalar, nc.gpsimd, nc.sync] if three_way else xq
            xts = []
            for m in range(8):
                t = xt_pool.tile([128, 512], FP16, name="xtc")
                qs[m % len(qs)].dma_start(
                    t[:], xt_ext[b, 128 * m:128 * (m + 1), 512 * mq:512 * (mq + 1)]
                )
                xts.append(t)
            return xts

        def emit_proj(b, mq, xts, fin=None):
            for ct in range(2):
                ps = ps_gen.tile([128, 512], F32, name="psqk", tag="gen")
                for m in range(8):
                    nc.tensor.matmul(
                        ps[:],
                        wqk_sb[m][:, 128 * ct:128 * (ct + 1)],
                        xts[m][:],
                        start=(m == 0),
                        stop=(m == 7),
                    )
                if ct == 0:
                    t = qk_pool.tile(
                        [128, 512], FP16, name=f"qk{ct}_{b}_{mq}", tag=f"qk{ct}_{b}_{mq}"
                    )
                    nc.vector.tensor_copy(t[:], ps[:])
                    qk_sb[(ct, b, mq)] = t
                    # the previous attention unit's deferred tail (final PV +
                    # normalize) lands here: the ct0 matmuls above fill the
                    # PE while that unit's last exp drains on ACT
                    if fin is not None:
                        fin()
                else:
                    # kT is consumed only as zero-padded per-head copies:
                    # K=128 scores matmuls run at full rate while 64-row
                    # quadrant matmuls measured ~1.5x slower per instruction.
                    # Copies on DVE: ACT is the attention pacer (exp), keep
                    # it exp-only.
                    for h in range(2):
                        kz = kz_pool.tile(
                            [128, 512], FP16, name=f"kz{h}_{b}_{mq}",
                            tag=f"kz{h}_{b}_{mq}",
                        )
                        nc.vector.memset(kz[64 - 64 * h:128 - 64 * h, :], 0.0)
                        nc.vector.tensor_copy(
                            kz[64 * h:64 * (h + 1), :],
                            ps[64 * h:64 * (h + 1), :],
                        )
                        kz_sb[(h, b, mq)] = kz
            for stl in range(4):
                ps = ps_gen.tile([128, 128], F32, name="psv", tag="gen")
                for m in range(8):
                    nc.tensor.matmul(
                        ps[:],
                        xts[m][:, 128 * stl:128 * (stl + 1)],
                        wv_sb[m][:],
                        start=(m == 0),
                        stop=(m == 7),
                    )
                kt = 4 * mq + stl
                # layout [vA(64) | 1 | vB(64) | 1]: ones column makes PV row 64
                # the softmax denominator; z dims land on rows 0..63 (DVE
                # partition ranges must start at 0/32/64/96, so z-rows-first)
                vt = v_pool.tile([128, 130], BF16, name=f"v_{b}_{kt}", tag=f"v_{b}_{kt}")
                nc.gpsimd.memset(vt[:, 64:65], 1.0)
                nc.gpsimd.memset(vt[:, 129:130], 1.0)
                nc.vector.tensor_copy(
                    vt[:].rearrange("p (g c) -> p g c", g=2)[:, :, 0:64],
                    ps[:].rearrange("p (g c) -> p g c", g=2),
                )
                v_sb[(b, kt)] = vt

        def emit_attn(b, mq, fin_prev=None, defer=False):
            nk = 4 * (mq + 1)
            # pz is allocated lazily at the first PV so pool-slot WAR
            # tracking stays consistent with deferred tails (the previous
            # unit's final PV may be emitted after this unit starts)
            pzc = []

            def get_pz():
                if not pzc:
                    pzc.append(ps_z.tile([128, 1024], F32, name="pz", tag="pz"))
                return pzc[0]

            def emit_scores_exp(g):
                # per head: 2 scores matmuls then immediately the exp, so ACT
                # starts on head 0 while the PE does head 1's scores. One
                # full-width exp per (group, head): ACT instructions have
                # ~700ns fixed overhead, so fewer/wider beats masked-region
                # skipping (measured +24us ACT when split per ktile).
                es = []
                for h in range(2):
                    psc = ps_sc.tile([128, 1024], F32, name="psc", tag="sc")
                    for kk in range(2):
                        kt = 2 * g + kk
                        mk, ktl = kt // 4, kt % 4
                        # diagonal tiles: q columns < 128*d are fully masked,
                        # so skip them in both scores and PV. The psum left
                        # unwritten holds stale-but-finite values whose exp
                        # is zeroed by the affine_select (q < 128d => masked).
                        d = kt - 4 * mq
                        q0 = 128 * d if d > 0 else 0
                        nc.tensor.matmul(
                            psc[:, 512 * kk + q0:512 * (kk + 1)],
                            kz_sb[(h, b, mk)][:, 128 * ktl:128 * (ktl + 1)],
                            qk_sb[(0, b, mq)][:, q0:512],
                            start=True,
                            stop=True,
                        )
                    e = e_pool.tile([128, 1024], BF16, name="etile")
                    nc.scalar.activation(e[:], psc[:], Exp)
                    es.append(e)
                for kk in range(2):
                    kt = 2 * g + kk
                    d = kt - 4 * mq
                    if d >= 0:  # diagonal: zero where k_local + 128*d > q_local
                        for h in range(2):
                            sl = es[h][:, 512 * kk:512 * (kk + 1)]
                            nc.gpsimd.affine_select(
                                out=sl,
                                in_=sl,
                                compare_op=mybir.AluOpType.is_ge,
                                fill=0.0,
                                base=-128 * d,
                                pattern=[[1, 512]],
                                channel_multiplier=-1,
                            )
                return es

            last_pe = [None]

            def emit_pv(g, es):
                pz = get_pz()
                for kk in range(2):
                    kt = 2 * g + kk
                    d = kt - 4 * mq
                    q0 = 128 * d if d > 0 else 0
                    vt = v_sb[(b, kt)]
                    for h in range(2):
                        last_pe[0] = nc.tensor.matmul(
                            pz[0:65, 512 * h + q0:512 * h + 512],
                            vt[:, 65 * h:65 * h + 65],
                            es[h][:, 512 * kk + q0:512 * (kk + 1)],
                            start=(kt == 0),
                            stop=(kt == nk - 1),
                        )

            # software pipeline: scores(g+1) is emitted BEFORE PV(g). The PE
            # queue is in-order, so otherwise PV(g)'s wait on exp(g) blocks
            # scores(g+1) that could already run - ~1us/group of PE idle.
            # The final PV + normalize are deferred (fin) so the NEXT
            # proj/unit's first matmuls can fill the last exp's latency; the
            # previous unit's fin lands right after this unit's first group.
            prev = None
            first = True
            for g in range(nk // 2):
                es = emit_scores_exp(g)
                if first:
                    if fin_prev is not None:
                        fin_prev()
                    first = False
                if prev is not None:
                    emit_pv(prev[0], prev[1])
                prev = (g, es)

            def fin():
                emit_tail(prev)
                return last_pe[0]

            def emit_tail(prev):
                emit_pv(prev[0], prev[1])
                pz = get_pz()
                # normalize: pz row 64 of each half = l. partition_broadcast
                # only reads from base partition 0, and DVE can't shift
                # partitions, so DMA the l row from psum partition 64 to
                # sbuf partition 0 first.
                zcp = nrm_pool.tile([65, 1024], F32, name="zcp", tag="zcp")
                nc.vector.tensor_copy(zcp[:, 0:512], pz[0:65, 0:512])
                nc.scalar.activation(
                    zcp[:, 512:1024],
                    pz[0:65, 512:1024],
                    mybir.ActivationFunctionType.Copy,
                )
                l0 = nrm_pool.tile([1, 1024], F32, name="l0", tag="l0")
                nc.gpsimd.dma_start(l0[:], zcp[64:65, :])
                rec = nrm_pool.tile([1, 1024], F32, name="rec", tag="rec")
                nc.vector.reciprocal_approx_fast(rec[:], l0[:])
                bc = nrm_pool.tile([64, 1024], F32, name="bc", tag="bc")
                nc.gpsimd.partition_broadcast(bc[:], rec[0:1, :])
                gcol = b * S + 512 * mq
                for h in range(2):
                    zt = z_pool.tile([64, 512], BF16, name="ztile")
                    nc.vector.tensor_mul(
                        zt[:],
                        zcp[0:64, 512 * h:512 * h + 512],
                        bc[:, 512 * h:512 * h + 512],
                    )
                    # routing: each rank's SECOND 512 rows (macros {1,3}) go
                    # to A2A_1 (fires after phase I); its first 512 rows
                    # ({0,2}) split across the tail collectives A2A_2a/2b.
                    w = 0
                    while w < 512:
                        gc = gcol + w
                        j, r = gc // CH, gc % CH
                        if r >= CH2:
                            buf, off = a2a_in1, r - CH2
                            ww = min(512 - w, CH - r)
                        elif r < CH4:
                            buf, off = a2a_in2a, r
                            ww = min(512 - w, CH4 - r)
                        else:
                            buf, off = a2a_in2b, r - CH4
                            ww = min(512 - w, CH2 - r)
                        nc.sync.dma_start(
                            buf[j, 64 * h:64 * (h + 1), off:off + ww],
                            zt[:, w:w + ww],
                        )
                        w += ww

            if defer:
                return fin, last_pe[0]
            fin()
            return None, last_pe[0]

        def emit_a2a(a2a_in, a2a_out):
            nc.gpsimd.collective_compute(
                "AllToAll",
                mybir.AluOpType.bypass,
                replica_groups=[list(range(NCORES))],
                ins=[a2a_in[:].opt()],
                outs=[a2a_out[:].opt()],
            )

        def emit_zrecv(a2a_out, tagp, width):
            # zrecv on sync: data-gated on the collective's completion
            # semaphore. Emitted as early as program order allows so the
            # reads stream during compute, not on the tail critical path.
            zrecv = []
            for ct in range(8):
                t = zr_pool.tile(
                    [128, width], BF16, name=f"zr{tagp}{ct}", tag=f"zr{tagp}{ct}"
                )
                nc.sync.dma_start(t[:], a2a_out[ct, :, :])
                zrecv.append(t)
            return zrecv

        def emit_oproj_qc(zrecv, row0, qc, after=None):
            # `after` pins the chunk's first matmul behind the intended
            # attention unit: the Tile scheduler otherwise hoists O-proj
            # (whose zrecv stationary load waits on a collective) ahead of
            # attention on the in-order PE queue, stalling it for the whole
            # collective (measured 49us).
            ob = ob_pool.tile([128, 1024], FP16, name="ob")
            pss = [
                ps_gen.tile([128, 512], F32, name="pso", tag="gen")
                for _ in range(2)
            ]
            first_mm = None
            for ct in range(8):
                for mh in range(2):
                    mm = nc.tensor.matmul(
                        pss[mh][:],
                        zrecv[ct][:, 128 * qc:128 * (qc + 1)],
                        ot_sb[ct][:, 512 * mh:512 * (mh + 1)],
                        start=(ct == 0),
                        stop=(ct == 7),
                    )
                    if first_mm is None:
                        first_mm = mm
                        if after is not None:
                            add_dep_helper(
                                mm.ins,
                                after.ins,
                                sync=False,
                                reason="keep O-proj chunk at its emission slot",
                            )
            for mh in range(2):
                nc.vector.tensor_copy(ob[:, 512 * mh:512 * (mh + 1)], pss[mh][:])
            r = row0 + 128 * qc
            nc.gpsimd.dma_start(out_ext[r:r + 128, :], ob[:])
            return first_mm

        # phase I: ALL projections + attention macros {1, 3} (the A2A_1 set:
        # each rank's second 512 output rows). {1,3} in phase I balances the
        # engines: proj (PE-only, no exp) interleaves with the exp-heavy big
        # macros, and it spreads the 16MB x stream over the whole phase.
        pend = {}
        pend[(0, 0)] = emit_xt(0, 0, three_way=True)
        pend[(0, 1)] = emit_xt(0, 1, three_way=True)
        fin = None
        for b in range(B):
            emit_proj(b, 0, pend.pop((b, 0)), fin=fin)
            fin = None
            emit_proj(b, 1, pend.pop((b, 1)))
            # batch 0's macro-2/3 loads precede any z write, so sync is
            # still safe to use for them (3-queue startup feed)
            pend[(b, 2)] = emit_xt(b, 2, three_way=(b == 0))
            pend[(b, 3)] = emit_xt(b, 3, three_way=(b == 0))
            f1, _ = emit_attn(b, 1, defer=True)
            emit_proj(b, 2, pend.pop((b, 2)), fin=f1)
            emit_proj(b, 3, pend.pop((b, 3)))
            if b + 1 < B:
                pend[(b + 1, 0)] = emit_xt(b + 1, 0)
                pend[(b + 1, 1)] = emit_xt(b + 1, 1)
            fin, _ = emit_attn(b, 3, defer=True)
        fin()  # flush attn(3,3)'s tail: A2A_1 needs its z writes now
        emit_a2a(a2a_in1, a2a_out1)
        for m in range(8):
            nc.scalar.dma_start(ot_sb[m][:], ot_ext[128 * m:128 * (m + 1), :])
        # zrecv for O-proj A is emitted BEFORE phase II: on the sync queue it
        # blocks (waiting on A2A_1) ahead of the first phase-II z writes, but
        # the enlarged z pool absorbs that backlog, and the reads complete
        # early in phase II — off the tail critical path.
        zrecv_a = emit_zrecv(a2a_out1, "a", CH2)
        # phase II: macros {0, 2} - ACT-bound (all exp, little proj work), so
        # the PE has slack: O-proj A chunks are interleaved between the later
        # attention units (by when A2A_1 and zrecv_a have certainly landed),
        # and its last chunk fills the PE while A2A_2a drains.
        units2 = [(bb, mqq) for bb in range(B) for mqq in (0, 2)]
        finII = None
        for i, (bb, mqq) in enumerate(units2):
            finII, lpe = emit_attn(bb, mqq, fin_prev=finII, defer=True)
        # phase II is ACT-paced, so O-proj A is NOT interleaved there: all
        # four chunks go after the final PV (anchored behind it) where they
        # fill the PE during the last unit's normalize chain and A2A_2a.
        # The A2A_2 triggers are emitted first on gpsimd so the chunks'
        # out-write issues can't delay them on the in-order Pool queue.
        link = finII()
        emit_a2a(a2a_in2a, a2a_out2a)
        emit_a2a(a2a_in2b, a2a_out2b)
        for qc in range(4):
            link = emit_oproj_qc(zrecv_a, CH2, qc, after=link)
        zrecv_b1 = emit_zrecv(a2a_out2a, "b1", CH4)
        zrecv_b2 = emit_zrecv(a2a_out2b, "b2", CH4)
        for qc in range(2):
            link = emit_oproj_qc(zrecv_b1, 0, qc, after=link)
        for qc in range(2):
            link = emit_oproj_qc(zrecv_b2, CH4, qc, after=link)

    nc.compile()
    return nc


_BUILT = {}


def _get_built(S):
    if S not in _BUILT:
        _BUILT[S] = build(S)
    return _BUILT[S]


def prep_inputs(x, Q, K, V, O):
    x = np.asarray(x, dtype=np.float32)
    Q = np.asarray(Q, dtype=np.float32)
    K = np.asarray(K, dtype=np.float32)
    V = np.asarray(V, dtype=np.float32)
    O = np.asarray(O, dtype=np.float32)
    xt = np.ascontiguousarray(np.transpose(x, (0, 2, 1))).astype(np.float16)  # [B, M, S]
    ot = np.ascontiguousarray(O.T).astype(ml_dtypes.bfloat16)  # [a, m], a = n*64+h
    in_maps = []
    for j in range(NCORES):
        hA, hB = 2 * j, 2 * j + 1
        wqk = np.ascontiguousarray(
            np.concatenate([Q[hA], Q[hB], K[hA], K[hB]], axis=0).T
        ).astype(np.float16)  # [1024, 256]
        wv = np.ascontiguousarray(
            np.concatenate([V[hA], V[hB]], axis=0).T
        ).astype(np.float16)  # [1024, 128]
        in_maps.append({"xt": xt, "wqk": wqk, "wv": wv, "ot": ot})
    return in_maps


def kernel(x, Q, K, V, O):
    global LAST_EXEC_TIME_NS
    x = np.asarray(x)
    S = x.shape[1]
    nc = _get_built(S)
    in_maps = prep_inputs(x, Q, K, V, O)
    trace = bool(int(os.environ.get("ATTN_TRACE", "0")))
    res = run_bass_kernel_spmd(nc, in_maps, list(range(NCORES)), trace=trace)
    LAST_EXEC_TIME_NS = res.exec_time_ns
    out = np.zeros((B, S, M), np.float32)
    half = S // 2
    for j in range(NCORES):
        b, hh = j // 2, j % 2
        out[b, hh * half:(hh + 1) * half, :] = res.results[j]["out"]
    return out

